# revision 8
# baseline (speedup 1.0000x reference)
"""Trainium2 Bass kernel for batched multi-head attention (B=8, N=M=C=1024,
H=16, D=64), data-parallel across 8 NeuronCores (one batch element per core).

v2: host-side prep of all layouts (pre-transposed bf16 q/k/v/target-mask and
weights; key mask folded into v), so the device kernel is pure compute:

Per-core dataflow (bf16 matmul inputs, f32 PSUM accumulate):
  - per head-pair j: project qh^T/kh^T (head-transposed) from pre-staged
    qbT/kbT; vh (natural, key-masked on host) with a trailing "key-indicator"
    column so the AV matmul also produces the softmax denominator.
  - QK^T row-packed two heads at a time (disjoint PE row groups, separate
    PSUM banks); exp on the scalar engine straight from PSUM with 1/sqrt(D)
    folded in; target mask as [128,512] bf16 DVE multiplies; AV with
    65-column lhsT -> numerator + denominator in one accumulation chain.
  - normalize via K=1 f32r ones-matmul broadcast of the denominator row +
    fast reciprocal + multiply; o-projection as K=128 accumulation chains
    with bo folded in as a K=1 ones matmul.
  - the PE instruction stream is software-pipelined: projection chains for
    head-pair j+1 are emitted between the QK^T groups of pair j, so the PE
    has filler work while the scalar engine computes exp (the scalar engine
    needs ~2x longer per score tile than the PE).
"""
import sys

sys.path.insert(0, "/opt/trn_rl_repo")

import numpy as np
import ml_dtypes

import concourse.bass as bass  # noqa: F401
import concourse.mybir as mybir
import concourse.bacc as bacc
import concourse.tile as tile
from concourse import bass_utils

B = 8
N = 1024   # queries
M = 1024   # keys
C = 1024   # model dim
H = 16
D = 64
NP = 8     # head pairs
P = 128
NB = 2     # n blocks of 512
SCALE = D ** -0.5

F32 = mybir.dt.float32
F32R = mybir.dt.float32r
BF16 = mybir.dt.bfloat16
I32 = mybir.dt.int32
MUL = mybir.AluOpType.mult
EXP = mybir.ActivationFunctionType.Exp
NPBF = ml_dtypes.bfloat16

_NC_CACHE = {}


def build_nc():
    nc = bacc.Bacc("TRN2", target_bir_lowering=False, debug=False, num_devices=1)

    qbT_d = nc.dram_tensor("qbT", [P, 8, N], BF16, kind="ExternalInput").ap()
    kbT_d = nc.dram_tensor("kbT", [P, 8, M], BF16, kind="ExternalInput").ap()
    vbT_d = nc.dram_tensor("vbT", [P, 8, M], BF16, kind="ExternalInput").ap()
    tmT_d = nc.dram_tensor("tmT", [P, 8, N], BF16, kind="ExternalInput").ap()
    mb_d = nc.dram_tensor("maskb", [P, 8], BF16, kind="ExternalInput").ap()
    wq_d = nc.dram_tensor("wq", [NP, P, 8, P], BF16, kind="ExternalInput").ap()
    wk_d = nc.dram_tensor("wk", [NP, P, 8, P], BF16, kind="ExternalInput").ap()
    wv_d = nc.dram_tensor("wv", [4, P, 8, 256], BF16, kind="ExternalInput").ap()
    wo_d = nc.dram_tensor("wo", [NP, P, C], BF16, kind="ExternalInput").ap()
    bob_d = nc.dram_tensor("bob", [1, C], BF16, kind="ExternalInput").ap()
    out_d = nc.dram_tensor("out", [N, C], F32, kind="ExternalOutput").ap()

    with tile.TileContext(nc) as tc:
        _body(tc, nc, qbT_d, kbT_d, vbT_d, tmT_d, mb_d, wq_d, wk_d, wv_d,
              wo_d, bob_d, out_d)
    nc.compile()
    return nc


def _body(tc, nc, qbT_d, kbT_d, vbT_d, tmT_d, mb_d, wq_d, wk_d, wv_d, wo_d,
          bob_d, out_d):
    from contextlib import ExitStack
    ctx = ExitStack()
    with ctx:
        persist = ctx.enter_context(tc.tile_pool(name="persist", bufs=1))
        wpool = ctx.enter_context(tc.tile_pool(name="wpool", bufs=2))
        ptpool = ctx.enter_context(tc.tile_pool(name="ptpool", bufs=10))
        xpool = ctx.enter_context(tc.tile_pool(name="xpool", bufs=2))
        opool = ctx.enter_context(tc.tile_pool(name="opool", bufs=2))
        spsum = ctx.enter_context(tc.tile_pool(name="spsum", bufs=2, space="PSUM"))
        avpsum = ctx.enter_context(tc.tile_pool(name="avpsum", bufs=2, space="PSUM"))

        # ---- persistent SBUF tensors ----
        qbT = persist.tile([P, 8, N], BF16)   # [p, cc, n] = q[n, cc*128+p]
        kbT = persist.tile([P, 8, M], BF16)
        vbT = persist.tile([P, 8, M], BF16)   # key-masked v, transposed
        tmT = persist.tile([P, 8, N], BF16)   # [p, mc, n] = tmask[n, mc*128+p]
        qhT = persist.tile([P, NP, N], BF16)  # [p, j, n] = qh[n, j*128+p]
        khT = persist.tile([P, NP, M], BF16)
        vha = persist.tile([P, NP, 8, 130], BF16)
        xn = persist.tile([P, NP, N], BF16)   # [p, j, n] = x_norm[n, j*128+p]
        wob = persist.tile([P, NP, C], BF16)  # [p, j, c2] = Wo[j*128+p, c2]
        maskb = persist.tile([P, 8], BF16)
        bob = persist.tile([1, C], BF16)
        onesb = persist.tile([1, P], BF16)
        nc.vector.memset(onesb[:], 1.0)
        ones_f = persist.tile([P, D], F32)
        nc.vector.memset(ones_f[:], 1.0)
        onesr = persist.tile([P, D], F32R)
        nc.vector.tensor_copy(onesr[:], ones_f[:])

        # ---- input DMAs (split into ~128KB chunks across queues) ----
        nc.sync.dma_start(out=maskb[:], in_=mb_d)
        nc.sync.dma_start(out=bob[:], in_=bob_d)

        def load_weights(j, split=1):
            wqb = wpool.tile([P, 8, P], BF16, tag="wq")
            wkb = wpool.tile([P, 8, P], BF16, tag="wk")
            s = 8 // split
            for i in range(split):
                cs = slice(i * s, (i + 1) * s)
                nc.sync.dma_start(out=wqb[:, cs, :], in_=wq_d[j, :, cs, :])
                nc.sync.dma_start(out=wkb[:, cs, :], in_=wk_d[j, :, cs, :])
            wvb = None
            if j % 2 == 0:
                wvb = wpool.tile([P, 8, 256], BF16, tag="wv")
                for i in range(2):
                    cs = slice(i * 4, (i + 1) * 4)
                    nc.sync.dma_start(out=wvb[:, cs, :], in_=wv_d[j // 2, :, cs, :])
            nc.sync.dma_start(out=wob[:, j, :], in_=wo_d[j])
            return wqb, wkb, wvb

        wqb0, wkb0, wvb0 = load_weights(0, split=4)
        # q/k first halves first (needed by proj(0) nb0)
        for cc in range(8):
            nc.sync.dma_start(out=qbT[:, cc, 0:512], in_=qbT_d[:, cc, 0:512])
            nc.sync.dma_start(out=kbT[:, cc, 0:512], in_=kbT_d[:, cc, 0:512])
        for cc in range(8):
            nc.sync.dma_start(out=qbT[:, cc, 512:1024], in_=qbT_d[:, cc, 512:1024])
            nc.sync.dma_start(out=kbT[:, cc, 512:1024], in_=kbT_d[:, cc, 512:1024])
        for cc in range(8):
            nc.sync.dma_start(out=vbT[:, cc, :], in_=vbT_d[:, cc, :])
        for cc in range(8):
            nc.sync.dma_start(out=tmT[:, cc, :], in_=tmT_d[:, cc, :])

        # ---- helpers ----
        def make_proj_fillers(j, wqb, wkb):
            tiles = {}

            def chain(which, nb):
                def run():
                    if nb not in tiles:
                        tiles[nb] = spsum.tile([P, 2, 512], F32, tag="sp",
                                               name=f"pj{j}_{nb}")
                    sp = tiles[nb]
                    ns = slice(nb * 512, (nb + 1) * 512)
                    half = 0 if which == "q" else 1
                    wb = wqb if which == "q" else wkb
                    src = qbT if which == "q" else kbT
                    dst = qhT if which == "q" else khT
                    for cc in range(8):
                        nc.tensor.matmul(sp[:, half, :], wb[:, cc, :],
                                         src[:, cc, ns],
                                         start=(cc == 0), stop=(cc == 7))
                    nc.vector.tensor_copy(dst[:, j, ns], sp[:, half, :])
                return run

            return [chain("q", 0), chain("k", 0), chain("q", 1), chain("k", 1)]

        def make_vproj_fillers(j, wvb):
            """v projection chains for pair (j, j+1), one filler per 2 m-chunks."""
            tiles = {}

            def chain(mg, mi2):
                def run():
                    if mg not in tiles:
                        tiles[mg] = spsum.tile([P, 2, 512], F32, tag="sp",
                                               name=f"pv{j}_{mg}")
                    pvv = tiles[mg].rearrange("p a b -> p (a b)").rearrange(
                        "p (m d) -> p m d", m=4)
                    for mi in (mi2, mi2 + 1):
                        mc = mg * 4 + mi
                        ms = slice(mc * P, (mc + 1) * P)
                        for cc in range(8):
                            nc.tensor.matmul(pvv[:, mi, :], vbT[:, cc, ms],
                                             wvb[:, cc, :],
                                             start=(cc == 0), stop=(cc == 7))
                        out_sl = vha[:, j:j + 2, mc, :].rearrange(
                            "p j (hx dd) -> p j hx dd", hx=2)[:, :, :, 0:64]
                        in_sl = pvv[:, mi, :].rearrange(
                            "p (j hx dd) -> p j hx dd", j=2, hx=2)
                        nc.vector.tensor_copy(out_sl, in_sl)
                    if mg == 1 and mi2 == 2:
                        for jx in (j, j + 1):
                            nc.vector.tensor_copy(vha[:, jx, :, 64], maskb[:])
                            nc.vector.tensor_copy(vha[:, jx, :, 129], maskb[:])
                return run

            return [chain(0, 0), chain(0, 2), chain(1, 0), chain(1, 2)]

        def qk_attn_g(j, nb, g, ptiles):
            """One QK^T group: scores for 2 m-chunks x 2 heads, exp, tm-mask."""
            ns = slice(nb * 512, (nb + 1) * 512)
            sp0 = spsum.tile([P, 2, 512], F32, tag="sp")
            sp1 = spsum.tile([P, 2, 512], F32, tag="sp")
            for mcx in range(2):
                mc = 2 * g + mcx
                ms = slice(mc * P, (mc + 1) * P)
                nc.tensor.matmul(sp0[:, mcx, :], khT[0:64, j, ms],
                                 qhT[0:64, j, ns], start=True, stop=True)
                nc.tensor.matmul(sp1[:, mcx, :], khT[64:128, j, ms],
                                 qhT[64:128, j, ns], start=True, stop=True)
            pt0 = ptpool.tile([P, 2, 512], BF16, tag="pt")
            pt1 = ptpool.tile([P, 2, 512], BF16, tag="pt")
            nc.scalar.activation(pt0[:], sp0[:], EXP, scale=SCALE)
            nc.scalar.activation(pt1[:], sp1[:], EXP, scale=SCALE)
            for mcx in range(2):
                mc = 2 * g + mcx
                tsl = tmT[:, mc, ns]
                nc.vector.tensor_tensor(pt0[:, mcx, :], pt0[:, mcx, :], tsl, MUL)
                nc.vector.tensor_tensor(pt1[:, mcx, :], pt1[:, mcx, :], tsl, MUL)
            ptiles[0][g] = pt0
            ptiles[1][g] = pt1

        av_tiles = {}

        def av_attn_half(j, nb, ptiles, half):
            """AV numerator+denominator accumulation, m-chunks half*4..half*4+3."""
            if half == 0:
                av_tiles[nb] = (avpsum.tile([65, 512], F32, tag="av", name="av0"),
                                avpsum.tile([65, 512], F32, tag="av", name="av1"))
            av0, av1 = av_tiles[nb]
            for mc in range(half * 4, half * 4 + 4):
                g, mcx = mc // 2, mc % 2
                nc.tensor.matmul(av0[:], vha[:, j, mc, 0:65],
                                 ptiles[0][g][:, mcx, :],
                                 start=(mc == 0), stop=(mc == 7))
                nc.tensor.matmul(av1[:], vha[:, j, mc, 65:130],
                                 ptiles[1][g][:, mcx, :],
                                 start=(mc == 0), stop=(mc == 7))

        def norm(j, nb):
            """denominator broadcast (K=1 f32r matmul) + reciprocal +
            multiply -> xn."""
            ns = slice(nb * 512, (nb + 1) * 512)
            av0, av1 = av_tiles[nb]
            for hx, av in ((0, av0), (1, av1)):
                xu = xpool.tile([65, 512], F32R, tag="xu")
                nc.scalar.copy(xu[:], av[:])
                bc = avpsum.tile([64, 512], F32, tag="bc", bufs=2)
                nc.tensor.matmul(bc[:], onesr[64:65, :], xu[64:65, :],
                                 start=True, stop=True)
                rc = xpool.tile([64, 512], F32, tag="rc")
                nc.vector.reciprocal_approx_fast(rc[:], bc[:])
                rows = slice(0, 64) if hx == 0 else slice(64, 128)
                nc.vector.tensor_tensor(xn[rows, j, ns],
                                        xu[0:64, :].bitcast(F32), rc[:], MUL)

        # ---- software-pipelined main loop ----
        f0 = make_proj_fillers(0, wqb0, wkb0)
        for f in f0:
            f()
        for f in make_vproj_fillers(0, wvb0):
            f()

        for j in range(NP):
            fillers = []
            if j + 1 < NP:
                wqb_n, wkb_n, wvb_n = load_weights(j + 1)
                fillers += make_proj_fillers(j + 1, wqb_n, wkb_n)
                if (j + 1) % 2 == 0:
                    fillers += make_vproj_fillers(j + 1, wvb_n)
            fi = 0

            pt_nb0 = [[None] * 4, [None] * 4]
            for g in range(4):
                qk_attn_g(j, 0, g, pt_nb0)
                if fi < len(fillers):
                    fillers[fi]()
                    fi += 1
            pt_nb1 = [[None] * 4, [None] * 4]
            qk_attn_g(j, 1, 0, pt_nb1)
            av_attn_half(j, 0, pt_nb0, 0)
            qk_attn_g(j, 1, 1, pt_nb1)
            av_attn_half(j, 0, pt_nb0, 1)
            qk_attn_g(j, 1, 2, pt_nb1)
            norm(j, 0)
            if fi < len(fillers):
                fillers[fi]()
                fi += 1
            qk_attn_g(j, 1, 3, pt_nb1)
            while fi < len(fillers):
                fillers[fi]()
                fi += 1
            av_attn_half(j, 1, pt_nb1, 0)
            av_attn_half(j, 1, pt_nb1, 1)
            norm(j, 1)

        # ---- o-projection (+ bias) ----
        for nch in range(8):
            nsl = slice(nch * P, (nch + 1) * P)
            for c2h in range(2):
                c2s = slice(c2h * 512, (c2h + 1) * 512)
                po = spsum.tile([P, 2, 512], F32, tag="sp")
                nc.tensor.matmul(po[:, 0, :], onesb[0:1, :], bob[0:1, c2s],
                                 start=True, stop=False)
                for j in range(NP):
                    nc.tensor.matmul(po[:, 0, :], xn[:, j, nsl], wob[:, j, c2s],
                                     start=False, stop=(j == NP - 1))
                ot = opool.tile([P, 512], F32, tag="ot")
                nc.scalar.copy(ot[:], po[:, 0, :])
                nc.sync.dma_start(out=out_d[nsl, c2s], in_=ot[:])


def _get_nc():
    if "nc" not in _NC_CACHE:
        _NC_CACHE["nc"] = build_nc()
    return _NC_CACHE["nc"]


def _prep_inputs(q, k, v, mask, target_mask, Wq, Wk, Wv, Wo, bo):
    """Host-side staging: transpose + bf16-cast into exact device layouts."""
    q = np.asarray(q, np.float32)
    k = np.asarray(k, np.float32)
    v = np.asarray(v, np.float32)
    mask = np.asarray(mask, np.int32)
    target_mask = np.asarray(target_mask, np.int32)

    def t_layout(x):
        # [N, C] -> [p, cc, n] with value x[n, cc*128+p]
        xT = np.ascontiguousarray(x.T).astype(NPBF)
        return np.ascontiguousarray(xT.reshape(8, P, -1).transpose(1, 0, 2))

    Wqb = np.asarray(Wq, np.float32).astype(NPBF)
    Wkb = np.asarray(Wk, np.float32).astype(NPBF)
    Wvb = np.asarray(Wv, np.float32).astype(NPBF)
    Wob = np.asarray(Wo, np.float32).astype(NPBF)
    shared = {
        # wq[j, p, cc, dj] = Wq[cc*128+p, j*128+dj]
        "wq": np.ascontiguousarray(
            Wqb.reshape(8, P, NP, P).transpose(2, 1, 0, 3)),
        "wk": np.ascontiguousarray(
            Wkb.reshape(8, P, NP, P).transpose(2, 1, 0, 3)),
        # wv[jp, p, cc, dd] = Wv[cc*128+p, jp*256+dd]
        "wv": np.ascontiguousarray(
            Wvb.reshape(8, P, 4, 256).transpose(2, 1, 0, 3)),
        # wo[j, p, c2] = Wo[j*128+p, c2]
        "wo": np.ascontiguousarray(Wob.reshape(NP, P, C)),
        "bob": np.ascontiguousarray(
            np.asarray(bo, np.float32).astype(NPBF).reshape(1, C)),
    }
    in_maps = []
    for b in range(B):
        vm = v[b] * mask[b].astype(np.float32)[:, None]
        m = {
            "qbT": t_layout(q[b]),
            "kbT": t_layout(k[b]),
            "vbT": t_layout(vm),
            "tmT": t_layout(target_mask[b].astype(np.float32)),
            "maskb": np.ascontiguousarray(
                mask[b].astype(np.float32).astype(NPBF).reshape(8, P).T),
        }
        m.update(shared)
        in_maps.append(m)
    return in_maps


def kernel(q, k, v, mask, target_mask, Wq, Wk, Wv, Wo, bo):
    nc = _get_nc()
    in_maps = _prep_inputs(q, k, v, mask, target_mask, Wq, Wk, Wv, Wo, bo)
    res = bass_utils.run_bass_kernel_spmd(nc, in_maps, core_ids=list(range(B)))
    out = np.stack([res.results[b]["out"] for b in range(B)], axis=0)
    return out.astype(np.float32)


def run_traced(q, k, v, mask, target_mask, Wq, Wk, Wv, Wo, bo, **trace_kwargs):
    """Like kernel() but with NTFF tracing; returns (out, BassKernelResults)."""
    nc = _get_nc()
    in_maps = _prep_inputs(q, k, v, mask, target_mask, Wq, Wk, Wv, Wo, bo)
    res = bass_utils.run_bass_kernel_spmd(nc, in_maps, core_ids=list(range(B)),
                                          trace=True, **trace_kwargs)
    out = np.stack([res.results[b]["out"] for b in range(B)], axis=0)
    return out.astype(np.float32), res
